# revision 44
# baseline (speedup 1.0000x reference)
"""NEAT layer kernel for Trainium2 (8 NeuronCores, pure data parallel).

Math (per reference): vals starts as x [B,64]; for each layer li with
(src, w, b): z = sum_k vals[:, src[n,k]] * w[n,k] + b[n]; out = sigmoid(5*z);
vals = concat(vals, out). Output = layer-3 out [B,10].

Strategy: the sparse gather+einsum is a chain of dense matmuls over the
accumulated node values (host-side the tiny (n,16) weights are scattered
into dense [src_block, dest_nodes] matrices). Nodes live on partitions,
batch on the free dim; sigmoid(5z+5b) is fused into the Activation that
drains each PSUM accumulation.

Changes vs the fp32r baseline (82 us -> 50.2 us), each from HW-trace
evidence:
- HAM warm-up burst (the big one). The PE clock-gate boots at K=4/8
  (1.2 GHz) and flips to 8/8 (2.4 GHz) only after one fully-busy
  aligned 3.4 us window; it flips back only after a fully-idle window.
  The ~95%-busy main loop can HOLD either state but never flip it, so
  the baseline ran its whole 65 us compute window cold (matmuls at
  (219+512)/1.2 = 610 ns; the model's 3 us ramp does not match cayman
  HW). Twelve dependency-free matmuls on zeroed scratch (>= 2 windows
  of span, guaranteeing one aligned busy window at any phase) latch
  2.4 GHz while the x DMAs stream in; warm 512-free matmuls run
  ~380 ns and the Act engine becomes the pacer.
- fp16 operands on the PE (numpy-checked end-to-end: rel err 4.7e-3 vs
  the 2e-2 gate; fp16's 11-bit mantissa ~ fp32r's 12). On this HW
  fp32r and fp16 stream at the same rate, so the win is not the PE -
  it halves LDWEIGHTS time and the x DMA bytes, and lifts the walrus
  fp32r partition-0-dst rule, enabling the q3 band packing below.
- Streamed input: weights first, then x in 4 group DMAs, so compute
  starts ~1.5 us after the DMA queue opens instead of serializing one
  2.4 MB blob (16.7 us of dead startup in the baseline).
- The z3 partial rows (p23[32:42]) are copied off PSUM by the DVE, not
  the Act engine (Act is the pacer; the copy cost it 680 ns/chunk).
  The PSUM bank tracker orders each copy after the same-bank A2 act,
  so the copy's PE wait is transitively dead - pruned post-hoc to keep
  every instruction within the one-sync-wait-slot HW limit, and the
  fused q3 matmul (Wh23 rows 0:32 + I10 rows 32:42, reading o2|partial
  [42 rows]) then waits only the DVE sem.
- z3 activations 4-packed: chunks 4g..4g+3 accumulate their 10-row z3
  into disjoint bands of one [40,512] PSUM bank via zero-padded [*,40]
  stationaries (matmul dst must start at partition 0/32/64), so one
  sigmoid Act serves 4 chunks ([10,512] acts cost the same ~670 ns as
  [128,512] ones - Act time is free-size cycles + fixed init).

Steady-state per-chunk engine busy: Act 3.25 activations (~2.2 us,
pacer), PE 7 matmuls (~1.7 us warm), DVE 1 copy (0.69 us). Act-paced
with PE holding an instruction backlog is the stable operating point:
structures that balance Act and PE (z23 2-packing cut Act to 1.84
us/chunk) flip pacing to a just-in-time PE whose cadence degrades
~240->340 ns/matmul (the 173 ns SBUF access latency stops pipelining),
measuring 55-56 us - worse, not better. Batch 65536 is split 8192 per
core; each core runs 16 chunks of 512.
"""

import sys

sys.path.insert(0, "/opt/trn_rl_repo")

import numpy as np

import concourse.bass as bass
import concourse.mybir as mybir
from concourse.tile import TileContext

BATCH = 65536
IN_DIM = 64
FAN_IN = 16
GAIN = 5.0
N_CORES = 8
BC = BATCH // N_CORES          # 8192 samples per core
CHUNK = 512
N_CHUNKS = BC // CHUNK         # 16

# Node index blocks in the accumulated `vals` array.
X_LO, X_HI = 0, 64             # x block
H0_LO, H0_HI = 64, 192         # out0 block
H1_LO, H1_HI = 192, 288        # out1 block
H2_LO, H2_HI = 288, 320        # out2 block

F16 = mybir.dt.float16
F32 = mybir.dt.float32

# Blob column layout (128 partitions x NBH fp16 words).
OFF_WX0 = 0        # [64,128] x->l0, duplicated on partitions 64..127
OFF_WX1 = 128      # [64,96]  x->l1, duplicated
OFF_WX23 = 224     # [64,64]  x->l2(cols 0..31)+l3 partial(cols 32..41), dup
OFF_WH01 = 288     # [128,96] out0->l1
OFF_WH023 = 384    # [128,64] out0->l2/l3 packed
OFF_WH123 = 448    # [96,64]  out1->l2/l3 packed
OFF_WQ = 512       # 4x [42,40] out2->l3 fused blocks: block k holds Wh23
                   # (rows 0..31) and I10 (rows 32..41, folds the z3
                   # partial) at cols 10k..10k+10, zero elsewhere (matmul
                   # dst base must be 0/32/64, so each chunk writes the
                   # full [40] band-bank, zeros outside its 10-row band)
OFF_B = 672        # 4 fp16 bias cols: 5*b0 (r0..127), 5*b1 (r0..95),
                   # 5*b2 (r0..31), 5*b3 tiled x4 (r0..39)
OFF_X = 676        # [128, 4096] xT: chunk c=(4g+k) at rows (k//2)*64..+64,
                   # cols g*1024 + (k%2)*512..+512
NBH = OFF_X + BC // 2          # 4772


def _scatter(dst: np.ndarray, src: np.ndarray, w: np.ndarray, lo: int, hi: int,
             col_off: int) -> None:
    """dst[src[n,k]-lo, n+col_off] += w[n,k] for src entries in [lo,hi)."""
    n, k = src.shape
    cols = np.repeat(np.arange(n, dtype=np.int64), k) + col_off
    s = src.ravel().astype(np.int64)
    v = w.ravel().astype(np.float64)
    m = (s >= lo) & (s < hi)
    np.add.at(dst, (s[m] - lo, cols[m]), v[m])


def _build_blob_base(inputs: dict) -> np.ndarray:
    """Weights+biases portion of the blob (x region left zero)."""
    Wx0 = np.zeros([64, 128], np.float64)
    Wx1 = np.zeros([64, 96], np.float64)
    Wx23 = np.zeros([64, 64], np.float64)
    Wh01 = np.zeros([128, 96], np.float64)
    Wh023 = np.zeros([128, 64], np.float64)
    Wh123 = np.zeros([96, 64], np.float64)
    Wh23 = np.zeros([32, 10], np.float64)

    # z23 packing: z2 dest nodes at cols 0..31, z3 partial at cols 32..41.
    _scatter(Wx0, inputs["src0"], inputs["w0"], X_LO, X_HI, 0)

    _scatter(Wx1, inputs["src1"], inputs["w1"], X_LO, X_HI, 0)
    _scatter(Wh01, inputs["src1"], inputs["w1"], H0_LO, H0_HI, 0)

    _scatter(Wx23, inputs["src2"], inputs["w2"], X_LO, X_HI, 0)
    _scatter(Wh023, inputs["src2"], inputs["w2"], H0_LO, H0_HI, 0)
    _scatter(Wh123, inputs["src2"], inputs["w2"], H1_LO, H1_HI, 0)

    _scatter(Wx23, inputs["src3"], inputs["w3"], X_LO, X_HI, 32)
    _scatter(Wh023, inputs["src3"], inputs["w3"], H0_LO, H0_HI, 32)
    _scatter(Wh123, inputs["src3"], inputs["w3"], H1_LO, H1_HI, 32)
    _scatter(Wh23, inputs["src3"], inputs["w3"], H2_LO, H2_HI, 0)

    blob = np.zeros([128, NBH], np.float16)
    for half in (slice(0, 64), slice(64, 128)):
        blob[half, OFF_WX0:OFF_WX0 + 128] = Wx0.astype(np.float16)
        blob[half, OFF_WX1:OFF_WX1 + 96] = Wx1.astype(np.float16)
        blob[half, OFF_WX23:OFF_WX23 + 64] = Wx23.astype(np.float16)
    blob[0:128, OFF_WH01:OFF_WH01 + 96] = Wh01.astype(np.float16)
    blob[0:128, OFF_WH023:OFF_WH023 + 64] = Wh023.astype(np.float16)
    blob[0:96, OFF_WH123:OFF_WH123 + 64] = Wh123.astype(np.float16)
    # WQ blocks duplicated at rows 64:106: stationary and moving operands
    # must share their base partition, and odd chunks' o2/partial live on
    # partitions 64:106 of the z23 2-pack.
    for k in range(4):
        c = OFF_WQ + 40 * k + 10 * k
        for r in (0, 64):
            blob[r:r + 32, c:c + 10] = Wh23.astype(np.float16)
            blob[r + 32:r + 42, c:c + 10] = np.eye(10, dtype=np.float16)

    def put_bias(rows: int, j: int, b: np.ndarray) -> None:
        blob[0:rows, OFF_B + j] = \
            (GAIN * np.asarray(b, np.float32)).astype(np.float16)

    put_bias(128, 0, inputs["b0"])
    put_bias(96, 1, inputs["b1"])
    # A2 covers both bands of the z23 2-pack: 5*b2 at rows 0:32 and 64:96.
    b2 = (GAIN * np.asarray(inputs["b2"], np.float32)).astype(np.float16)
    blob[0:32, OFF_B + 2] = b2
    blob[64:96, OFF_B + 2] = b2
    put_bias(40, 3, np.tile(np.asarray(inputs["b3"], np.float32), 4))
    return blob


def build_nc() -> bass.Bass:
    nc = bass.Bass()
    blob_d = nc.declare_dram_parameter("blob", [128, NBH], F16, isOutput=False)
    # Three output tensors so results stream out overlapped with compute:
    # packs 0+1 in one DMA, packs 2 and 3 separately (the pack-3 DMA is
    # the kernel tail - half the transfer shortens the drain). Separate
    # DRAM tensors avoid WAW waits between the DMAs, and 5 input + 3
    # output DMAs exactly fit the 8 HW DMA queues without wrapping (a
    # wrapped queue adds a FIFO wait on top of the Act wait - two lanes).
    yTs = [nc.declare_dram_parameter("yT0", [40, 2 * CHUNK], F32,
                                     isOutput=True)]
    yTs += [nc.declare_dram_parameter(f"yT{k}", [40, CHUNK], F32,
                                      isOutput=True) for k in (1, 2)]

    SIG = mybir.ActivationFunctionType.Sigmoid

    with TileContext(nc) as tc:
        with (
            tc.tile_pool(name="persist", bufs=1) as pp,
            tc.tile_pool(name="ps0", bufs=2, space="PSUM") as pz0,
            tc.tile_pool(name="ps1", bufs=2, space="PSUM") as pz1,
            tc.tile_pool(name="ps23", bufs=2, space="PSUM") as pz23,
            tc.tile_pool(name="psq", bufs=2, space="PSUM") as pzq,
        ):
            wsb = pp.tile([128, OFF_X], F16)
            xsb = pp.tile([128, BC // 2], F16)
            warm_sb = pp.tile([128, 1], F16)
            scratch = pp.tile([128, CHUNK], F16)
            o0_sb = pp.tile([128, BC], F16)
            o1_sb = pp.tile([96, BC], F16)
            # out2 on partitions 0..31, z3 partial (DVE-copied) on 32..41.
            o2_sb = pp.tile([42, BC], F16)
            o3_sb = pp.tile([40, 4 * CHUNK], F32)

            def bias(hi, j):
                c = OFF_B + j
                return wsb[0:hi, c:c + 1]

            # Weights first, then x groups, all on one sync DMA queue:
            # every consumer waits a single increasing sem value on one
            # lane (one sync-wait slot per instruction on HW).
            nc.sync.dma_start(out=wsb[:], in_=blob_d[:, 0:OFF_X])
            # Warmup: puts the weights-DMA wait into the Act engine clock
            # so later ACTs' bias deps are elided (1 wait slot each).
            nc.scalar.copy(warm_sb[:], wsb[:, OFF_B:OFF_B + 1])
            for g in range(4):
                nc.sync.dma_start(
                    out=xsb[:, g * 1024:(g + 1) * 1024],
                    in_=blob_d[:, OFF_X + g * 1024:OFF_X + (g + 1) * 1024])

            # HAM warm-up burst. The PE's clock gate defaults to K=4/8
            # (1.2 GHz) and un-throttles to 2.4 GHz only after a ~3.4 us
            # window of near-100% PE occupancy; conversely it re-throttles
            # only after a ~3.4 us idle window. The pipelined main loop is
            # ~95% busy - enough to HOLD either state, not enough to flip
            # it - so without this burst the whole kernel runs at 1.2 GHz
            # (measured: 512-free matmuls at (219+512)/1.2 = 610 ns for
            # the full 65 us window, fp32r and fp16 alike). 16 dependency-
            # free back-to-back matmuls on zeroed scratch run while the x
            # DMAs stream in, spanning >2 windows to guarantee one fully
            # aligned busy window, latching 2.4 GHz for the real work.
            # DVE memset: the DVE queue is empty at preamble end, while
            # GpSimd first drains the framework's own const-ap memsets -
            # this starts the burst (and the HAM latch) ~1.3 us earlier.
            nc.vector.memset(scratch[:], 0.0)
            warm_ps = pz0.tile([128, CHUNK], F32, name="p0")
            for _ in range(12):
                nc.tensor.matmul(warm_ps[:], scratch[0:128, 0:128],
                                 scratch[:], start=True, stop=True)

            def xap(c):
                g, k = divmod(c, 4)
                rb = (k // 2) * 64
                col = g * 1024 + (k % 2) * 512
                return rb, xsb[rb:rb + 64, col:col + CHUNK]

            # Software pipeline, layer-major with DEEP stagger: at step t
            # chunk t runs layer 0, chunk t-1 layer 1, chunk t-3 the z23
            # matmuls, chunk t-5 the layer-3 matmul. The extra slack (deps
            # produced >=2 steps before use, vs the minimum 1) keeps the
            # PE instruction queue backlogged: a backlogged PE overlaps
            # the 173 ns SBUF access latency across matmuls (~240 ns
            # cadence); a just-in-time PE serializes it (~340 ns) and
            # becomes the pacer. Per-layer PSUM pools (1 bank x 2 bufs
            # each = 8 banks).
            # Wait-slot audit (1 sync wait max per matmul/act): M0 waits
            # the DMA queue lane; M1b waits Act(A0); the z23 o1-matmul
            # waits Act(A1); q3 waits DVE(copy) after pruning below; every
            # act waits its producing matmul; all other deps sit below the
            # engines' observed vector clocks.
            q3s = {}
            for t in range(N_CHUNKS + 3):
                c0, c1, c2, c3 = t, t - 1, t - 2, t - 3
                if c0 < N_CHUNKS:
                    cs = slice(c0 * CHUNK, (c0 + 1) * CHUNK)
                    rb, X = xap(c0)
                    p0 = pz0.tile([128, CHUNK], F32, name="p0")
                    nc.tensor.matmul(p0[:], wsb[rb:rb + 64, 0:128], X,
                                     start=True, stop=True)
                    nc.scalar.activation(o0_sb[:, cs], p0[:], SIG,
                                         bias=bias(128, 0), scale=GAIN)
                if 0 <= c1 < N_CHUNKS:
                    cs = slice(c1 * CHUNK, (c1 + 1) * CHUNK)
                    rb, X = xap(c1)
                    p1 = pz1.tile([96, CHUNK], F32, name="p1")
                    nc.tensor.matmul(p1[:], wsb[rb:rb + 64, 128:224], X,
                                     start=True, stop=False)
                    nc.tensor.matmul(p1[:], wsb[0:128, 288:384],
                                     o0_sb[:, cs], start=False, stop=True)
                    nc.scalar.activation(o1_sb[:, cs], p1[:], SIG,
                                         bias=bias(96, 1), scale=GAIN)
                if 0 <= c2 < N_CHUNKS:
                    cs = slice(c2 * CHUNK, (c2 + 1) * CHUNK)
                    rb, X = xap(c2)
                    p23 = pz23.tile([64, CHUNK], F32, name="p23")
                    nc.tensor.matmul(p23[:], wsb[rb:rb + 64, 224:288], X,
                                     start=True, stop=False)
                    nc.tensor.matmul(p23[:], wsb[0:128, 384:448],
                                     o0_sb[:, cs], start=False, stop=False)
                    nc.tensor.matmul(p23[:], wsb[0:96, 448:512],
                                     o1_sb[:, cs], start=False, stop=True)
                    nc.scalar.activation(o2_sb[0:32, cs], p23[0:32, :], SIG,
                                         bias=bias(32, 2), scale=GAIN)
                    nc.vector.tensor_copy(o2_sb[32:42, cs], p23[32:42, :])
                if 0 <= c3 < N_CHUNKS:
                    cs = slice(c3 * CHUNK, (c3 + 1) * CHUNK)
                    g, k = divmod(c3, 4)
                    if k == 0:
                        q3s[g] = pzq.tile([40, CHUNK], F32, name="q3")
                    q3 = q3s[g]
                    wq = OFF_WQ + 40 * k
                    nc.tensor.matmul(q3[:], wsb[0:42, wq:wq + 40],
                                     o2_sb[:, cs],
                                     start=(k == 0), stop=(k == 3))
                    if k == 3:
                        oc = slice(g * CHUNK, (g + 1) * CHUNK)
                        nc.scalar.activation(o3_sb[:, oc], q3[:], SIG,
                                             bias=bias(40, 3), scale=GAIN)
                        if g == 1:
                            nc.sync.dma_start(out=yTs[0][:],
                                              in_=o3_sb[:, 0:2 * CHUNK])
                        elif g >= 2:
                            nc.sync.dma_start(out=yTs[g - 1][:],
                                              in_=o3_sb[:, oc])

    # HW allows one sync wait per instruction; prune transitively-redundant
    # waits Tile emits (it is not transitively minimal across engines).
    #
    # DVE copies: the PSUM bank-overlap tracker serializes each copy after
    # the A2 act reading the same p23 bank (Act wait) on top of the RAW
    # dep on the producing matmul (PE wait). The Act wait dominates: A2
    # itself waits that same stop-matmul, so drop the PE lane.
    for i in nc.all_instructions():
        if type(i).__name__ == "InstTensorCopy" and i.sync_info and \
                len(i.sync_info.on_wait) > 1:
            si = i.sync_info
            acts = [w for w in si.on_wait if w.ant_name.startswith("Activation")]
            if acts and any(w.ant_name.startswith("PE") for w in si.on_wait):
                si.on_wait = acts
                i.sync_info = si
    # q3 matmuls read o2_sb rows 0:32 (written by Act A2) and rows 32:42
    # (the z3 partial, written by the DVE copy). The DVE copy itself runs
    # after A2 (pruned above to wait exactly A2's sem), so the DVE wait
    # dominates the Act wait: drop the Act lane.
    for i in nc.all_instructions():
        if type(i).__name__ == "InstMatmult" and i.sync_info and \
                len(i.sync_info.on_wait) > 1:
            si = i.sync_info
            dve = [w for w in si.on_wait if w.ant_name.startswith("DVE")]
            if dve and any(w.ant_name.startswith("Activation")
                           for w in si.on_wait):
                si.on_wait = [w for w in si.on_wait
                              if not w.ant_name.startswith("Activation")]
                i.sync_info = si
    # The teardown Drain waits on every engine's final sem value, but HW
    # allows one sync wait per instruction. The LAST out DMA's completion
    # transitively dominates them all (it starts after the final ACT,
    # which waited the final matmul chain; all DMAs share one FIFO queue
    # so earlier DMAs finish before it). Prune the drain to that lane.
    for i in nc.all_instructions():
        if type(i).__name__ == "InstDrain" and i.sync_info and \
                len(i.sync_info.on_wait) > 1:
            dma_lane = None
            for j in nc.all_instructions():
                if type(j).__name__ == "InstDMACopy" and j.sync_info:
                    for u in j.sync_info.on_update:
                        if j.sync_info.on_wait:
                            dma_lane = u.ant_name
            si = i.sync_info
            si.on_wait = [w for w in si.on_wait if w.ant_name == dma_lane]
            i.sync_info = si
    return nc


def make_in_maps(inputs: dict) -> list[dict]:
    base = _build_blob_base(inputs)
    x = np.asarray(inputs["x"], dtype=np.float32).astype(np.float16)
    in_maps = []
    for i in range(N_CORES):
        b = base.copy()
        xT = np.ascontiguousarray(x[i * BC:(i + 1) * BC, :].T)  # [64, 8192]
        for c in range(N_CHUNKS):
            g, k = divmod(c, 4)
            rb = (k // 2) * 64
            col = OFF_X + g * 1024 + (k % 2) * 512
            b[rb:rb + 64, col:col + CHUNK] = xT[:, c * CHUNK:(c + 1) * CHUNK]
        in_maps.append({"blob": b})
    return in_maps


def assemble_core(res: dict) -> np.ndarray:
    """[BC, 10] from one core's yT0..2 (4-packed z3 partition bands)."""
    y = np.empty((BC, 10), np.float32)
    for g in range(4):
        if g < 2:
            yt = res["yT0"][:, g * CHUNK:(g + 1) * CHUNK]
        else:
            yt = res[f"yT{g - 1}"]
        for k in range(4):
            c = 4 * g + k
            y[c * CHUNK:(c + 1) * CHUNK, :] = yt[10 * k:10 * k + 10, :].T
    return y


def assemble_output(results: list[dict]) -> np.ndarray:
    y = np.empty((BATCH, 10), np.float32)
    for i in range(N_CORES):
        y[i * BC:(i + 1) * BC, :] = assemble_core(results[i])
    return y


def kernel(**inputs: np.ndarray) -> np.ndarray:
    from concourse.bass_utils import run_bass_kernel_spmd

    nc = build_nc()
    in_maps = make_in_maps(inputs)
    res = run_bass_kernel_spmd(nc, in_maps, list(range(N_CORES)))
    return assemble_output(res.results)
